# revision 21
# baseline (speedup 1.0000x reference)
"""Trainium2 Bass kernel for the vq_codebook block (nn_Block_89824946029265).

Computation (per token t, D=1024, V=64):
  g_t  = mean_d sigmoid(x * gate)
  x1   = (1-g) x + g emb_weight[idx]
  logits = x1 @ head_weight.T            # [*, 64]
  soft_prob = softmax(logits)
  x2   = (1-sg) x1 + sg (soft_prob @ emb_weight),  sg = mean_d sigmoid(x1*soft_gate)
  new_idx = argmax(logits)

Reformulation used on device (W2 := emb_weight @ head_weight.T, host f64):
  xs      = (1-g) x
  logits  = xs @ head_weight.T + g*W2[idx]            (exact f32 path -> argmax safe)
  x1^T    = xs^T + (g*one_hot)^T.T-matmul(emb_weight) (PSUM, f32r, feeds sigma2 only)
  sg      = mean sigma(softgate ⊙ x1^T)               (ACT on transposed blocks)
  x2      = (1-sg) xs + [(1-sg)*g*one_hot + sg*soft_prob] @ emb_weight   (f32r)

Sharding: pure data parallel, batch dim B=8 -> one batch row (4096 tokens)
per NeuronCore.
"""

import numpy as np

import concourse.bass as bass
import concourse.mybir as mybir
from concourse import tile
from concourse.bass_utils import run_bass_kernel_spmd
from concourse.tile_rust import add_dep_helper


def _dep(later, earlier):
    """Order `later` after `earlier` (PSUM bank first-writer ordering)."""
    add_dep_helper(later.ins, earlier.ins, reason="psum bank start ordering")

F32 = mybir.dt.float32
F32R = mybir.dt.float32r
I32 = mybir.dt.int32
BF16 = mybir.dt.bfloat16
AX = mybir.AxisListType
OP = mybir.AluOpType
AF = mybir.ActivationFunctionType

N_CORES = 8
B, T, D, V = 8, 4096, 1024, 64
TOK = B * T // N_CORES          # 4096 tokens per core
P = 128                         # partitions / tokens per tile
NT = TOK // P                   # 32 tiles per core
NK = D // P                     # 8 D-slices
NPAIR = NT // 2                 # 16 pairs

_CACHE = {}


def r(ap):
    """View an fp32 AP as float32r (fast PE path, fp22 truncation)."""
    return ap.bitcast(F32R)


def build_kernel():
    nc = bass.Bass()

    # ---- per-core I/O ----
    x_in = nc.declare_dram_parameter("x", [TOK, D], F32, isOutput=False)
    idxf_in = nc.declare_dram_parameter("idxf", [P, NT], F32, isOutput=False)
    wemb_in = nc.declare_dram_parameter("w_emb", [V, D], F32, isOutput=False)
    hwt_in = nc.declare_dram_parameter("hw_t", [P, NK * V], F32, isOutput=False)
    w2_in = nc.declare_dram_parameter("w2", [V, V], F32, isOutput=False)
    gate_in = nc.declare_dram_parameter("gate_rep", [P, D], F32, isOutput=False)
    sgc_in = nc.declare_dram_parameter("sgate_cols", [P, NK], F32, isOutput=False)
    iota_in = nc.declare_dram_parameter("iota64", [P, P], F32, isOutput=False)
    ones_in = nc.declare_dram_parameter("ones", [P, 1], BF16, isOutput=False)
    wembb_in = nc.declare_dram_parameter("w_emb_b", [V, D], BF16, isOutput=False)
    wembbl_in = nc.declare_dram_parameter("w_emb_bl", [V, D], BF16, isOutput=False)
    ident_in = nc.declare_dram_parameter("ident", [P, P], F32, isOutput=False)

    x_out = nc.declare_dram_parameter("x_out", [TOK, D], F32, isOutput=True)
    idx_out = nc.declare_dram_parameter("idx_out", [P, NT], I32, isOutput=True)

    with tile.TileContext(nc) as tc:
        with (
            tc.tile_pool(name="const", bufs=1) as cpool,
            tc.tile_pool(name="io", bufs=3) as iopool,
            tc.tile_pool(name="work", bufs=3) as wpool,
            tc.tile_pool(name="pairw", bufs=3) as ppool,
            tc.tile_pool(name="small", bufs=4) as spool,
            tc.tile_pool(name="ps_big", bufs=2, space="PSUM") as ps_big,
            tc.tile_pool(name="ps_lg", bufs=2, space="PSUM") as ps_lg,
            tc.tile_pool(name="ps_comb", bufs=1, space="PSUM") as ps_comb,
        ):
            # ---- preload constants ----
            w_emb = cpool.tile([V, D], F32)
            wembB = cpool.tile([V, D], BF16)
            wembBlo = cpool.tile([V, D], BF16)
            hw_t = cpool.tile([P, NK * V], F32)
            w2 = cpool.tile([V, V], F32)
            gate_rep = cpool.tile([P, D], F32)
            sgate_cols = cpool.tile([P, NK], F32)
            iota128 = cpool.tile([P, P], F32)
            ones = cpool.tile([P, 1], BF16)
            ident = cpool.tile([P, P], F32)
            idxf = cpool.tile([P, NT], F32)
            idx_sb = cpool.tile([P, NT], I32)

            nc.sync.dma_start(w_emb[:], wemb_in[:])
            nc.sync.dma_start(wembB[:], wembb_in[:])
            nc.sync.dma_start(wembBlo[:], wembbl_in[:])
            nc.sync.dma_start(hw_t[:], hwt_in[:])
            nc.sync.dma_start(w2[:], w2_in[:])
            nc.sync.dma_start(gate_rep[:], gate_in[:])
            nc.sync.dma_start(sgate_cols[:], sgc_in[:])
            nc.sync.dma_start(iota128[:], iota_in[:])
            nc.sync.dma_start(ones[:], ones_in[:])
            nc.sync.dma_start(ident[:], ident_in[:])
            nc.sync.dma_start(idxf[:], idxf_in[:])

            for i in range(NT):
                s0 = i * P
                # ---------- per-tile PSUM ----------
                x1t = ps_big.tile([P, D], F32)        # xs^T then x1^T (2 banks)
                # lgbank regions: logits 0:64 | ohgT 64:192 | sgrow(p0) 192:320
                #                 | pcombT 320:448 | sgcol 448
                lgbank = ps_lg.tile([P, 512], F32)
                cmb_t = ps_comb.tile([P, D], F32)     # comb (2 banks, late phase)

                # ---------- token-space gate 1 ----------
                x_t = iopool.tile([P, D], F32, tag="x")
                nc.sync.dma_start(x_t[:], x_in[s0:s0 + P, :])

                m1 = wpool.tile([P, D], F32, tag="m1")
                nc.gpsimd.tensor_tensor(m1[:], x_t[:], gate_rep[:], OP.mult)

                s1 = wpool.tile([P, D], BF16, tag="s1")
                gsum = spool.tile([P, 1], F32, tag="gsum")
                nc.scalar.activation(s1[:], m1[:], AF.Tanh, accum_out=gsum[:])

                gcol = spool.tile([P, 1], F32, tag="gcol")
                g1m = spool.tile([P, 1], F32, tag="g1m")
                nc.gpsimd.tensor_scalar(gcol[:], gsum[:], 0.5 / D, 0.5, OP.mult, OP.add)
                nc.gpsimd.tensor_scalar(g1m[:], gsum[:], -0.5 / D, 0.5, OP.mult, OP.add)

                # xs = (1-g) * x  (GPSIMD single-input ~line rate)
                xs = wpool.tile([P, D], F32, tag="xs")
                nc.gpsimd.tensor_scalar(xs[:], x_t[:], g1m[:], None, OP.mult)

                # g-scaled one-hot, padded to 128 cols (64: zeros)
                ohg = spool.tile([P, P], F32, tag="ohg")
                nc.gpsimd.tensor_scalar(
                    ohg[:], iota128[:], idxf[:, i:i + 1], gcol[:],
                    OP.is_equal, OP.mult)
                lg_first = nc.tensor.matmul(
                    lgbank[:, 64:64 + P], ohg[:], ident[:],
                    is_transpose=True, start=True, stop=False,
                    skip_group_check=True)
                ohgT = spool.tile([V, P], F32, tag="ohgT")
                nc.vector.tensor_copy(ohgT[:], lgbank[0:V, 64:64 + P])
                ohgTb = spool.tile([V, P], BF16, tag="ohgTb")
                nc.vector.tensor_copy(ohgTb[:], lgbank[0:V, 64:64 + P])

                # ---------- xs^T transposes + copy + emb accumulation ----------
                bank_first = [None, None]
                for k in range(NK):
                    b = k // 4
                    t = nc.tensor.matmul(
                        x1t[:, k * P:(k + 1) * P],
                        xs[:, k * P:(k + 1) * P], ident[:],
                        is_transpose=True, start=(k % 4 == 0), stop=False,
                        skip_group_check=True)
                    if k % 4 == 0:
                        bank_first[b] = t
                    else:
                        _dep(t, bank_first[b])

                xsT = ppool.tile([P, D], F32, tag="xsT")
                nc.scalar.copy(xsT[:, 0:384], x1t[:, 0:384])
                nc.vector.tensor_copy(xsT[:, 384:D], x1t[:, 384:D])

                # x1^T = xs^T + g*emb^T   (bf16 matmuls, feeds sigma2 only)
                for k in range(NK):
                    nc.tensor.matmul(
                        x1t[:, k * P:(k + 1) * P],
                        wembB[:, k * P:(k + 1) * P], ohgTb[:],
                        start=False, stop=(k % 4 == 3), skip_group_check=True)

                # ---------- sigma2 (tanh form) ----------
                s2t = ppool.tile([P, D], BF16, tag="s2t")
                for k in range(NK):
                    nc.scalar.activation(
                        s2t[:, k * P:(k + 1) * P],
                        x1t[:, k * P:(k + 1) * P],
                        AF.Tanh, scale=sgate_cols[:, k:k + 1])

                # ---------- logits ----------
                lg = lgbank[:, 0:V]
                for k in range(NK):
                    mm = nc.tensor.matmul(
                        lg, xsT[:, k * P:(k + 1) * P],
                        hw_t[:, k * V:(k + 1) * V],
                        start=False, stop=False, skip_group_check=True)
                    if k == 0:
                        _dep(mm, lg_first)
                nc.tensor.matmul(lg, ohgT[:], w2[:],
                                 start=False, stop=True, skip_group_check=True)

                # sg sums (partition 0, cols 192:320)
                for k in range(NK):
                    mm = nc.tensor.matmul(
                        lgbank[0:1, 192:192 + P],
                        ones[:], s2t[:, k * P:(k + 1) * P],
                        start=False, stop=(k == NK - 1),
                        skip_group_check=True)
                    if k == 0:
                        _dep(mm, lg_first)
                sgrow = spool.tile([1, P], F32, tag="sgrow")
                nc.vector.tensor_copy(sgrow[:], lgbank[0:1, 192:192 + P])
                t_sg = nc.tensor.matmul(lgbank[:, 448:449], sgrow[0:1, :],
                                        ident[0:1, 0:1],
                                        is_transpose=True, start=False, stop=True,
                                        skip_group_check=True)
                _dep(t_sg, lg_first)
                sgsum = spool.tile([P, 1], F32, tag="sgsum")
                nc.vector.tensor_copy(sgsum[:], lgbank[:, 448:449])

                # ---------- softmax + argmax ----------
                lgs = spool.tile([P, V], F32, tag="lgs")
                nc.vector.tensor_copy(lgs[:], lg)
                mx8 = spool.tile([P, 8], F32, tag="mx8")
                nc.vector.max(mx8[:], lgs[:])
                ix8 = spool.tile([P, 8], mybir.dt.uint32, tag="ix8")
                nc.vector.max_index(ix8[:], mx8[:], lgs[:])
                nc.vector.tensor_copy(idx_sb[:, i:i + 1], ix8[:, 0:1].bitcast(I32))

                nmx = spool.tile([P, 1], F32, tag="nmx")
                nc.gpsimd.tensor_scalar(nmx[:], mx8[:, 0:1], -1.0, None, OP.mult)
                exps = spool.tile([P, V], F32, tag="exps")
                sume = spool.tile([P, 1], F32, tag="sume")
                nc.scalar.activation(exps[:], lg, AF.Exp, bias=nmx[:],
                                     accum_out=sume[:])
                rcp = spool.tile([P, 1], F32, tag="rcp")
                nc.vector.reciprocal(rcp[:], sume[:])

                sgv = spool.tile([P, 1], F32, tag="sgv")
                nc.gpsimd.tensor_scalar(sgv[:], sgsum[:], 0.5 / D, 0.5,
                                        OP.mult, OP.add)
                alpha = spool.tile([P, 1], F32, tag="alpha")
                nc.gpsimd.tensor_tensor(alpha[:], sgv[:], rcp[:], OP.mult)
                beta = spool.tile([P, 1], F32, tag="beta")
                nc.gpsimd.tensor_scalar(beta[:], sgsum[:], -0.5 / D, 0.5,
                                        OP.mult, OP.add)

                spr = spool.tile([P, V], F32, tag="spr")
                nc.gpsimd.tensor_scalar(spr[:], exps[:], alpha[:], None, OP.mult)
                pcomb = spool.tile([P, V], F32, tag="pcomb")
                nc.vector.scalar_tensor_tensor(pcomb[:], ohg[:, 0:V], beta[:], spr[:],
                                               OP.mult, OP.add)
                t_pc = nc.tensor.matmul(
                    lgbank[0:V, 320:320 + P], pcomb[:], ident[:],
                    is_transpose=True, start=False, stop=True,
                    skip_group_check=True)
                _dep(t_pc, lg_first)
                pcombT = spool.tile([V, P], F32, tag="pcombT")
                nc.vector.tensor_copy(pcombT[:], lgbank[0:V, 320:320 + P])
                pcT_hi = spool.tile([V, P], BF16, tag="pcT_hi")
                nc.vector.tensor_copy(pcT_hi[:], pcombT[:])
                pcT_lo = spool.tile([V, P], BF16, tag="pcT_lo")
                nc.vector.tensor_tensor(pcT_lo[:], pcombT[:], pcT_hi[:],
                                        OP.subtract)

                # ---------- comb + x2 ----------
                x2 = iopool.tile([P, D], F32, tag="x2")
                for h in range(2):
                    cmb = cmb_t[:, h * 512:(h + 1) * 512]
                    wh = slice(h * 512, (h + 1) * 512)
                    nc.tensor.matmul(cmb, pcT_hi[:], wembB[:, wh],
                                     start=True, stop=False,
                                     skip_group_check=True)
                    nc.tensor.matmul(cmb, pcT_lo[:], wembB[:, wh],
                                     start=False, stop=False,
                                     skip_group_check=True)
                    nc.tensor.matmul(cmb, pcT_hi[:], wembBlo[:, wh],
                                     start=False, stop=True,
                                     skip_group_check=True)
                nc.vector.scalar_tensor_tensor(
                    x2[:], xs[:], beta[:], cmb_t[:], OP.mult, OP.add)
                nc.sync.dma_start(x_out[s0:s0 + P, :], x2[:])

            nc.sync.dma_start(idx_out[:], idx_sb[:])

    return nc


def _prep_inputs(x, idx, emb_weight, head_weight, gate, soft_gate):
    """Host-side prep: shard + derived tensors. Returns per-core input maps."""
    x = np.ascontiguousarray(np.asarray(x, dtype=np.float32).reshape(B * T, D))
    idx = np.asarray(idx)
    idx_c = np.clip(idx, 0, None).astype(np.float32).reshape(B * T)
    emb_weight = np.asarray(emb_weight, dtype=np.float32)
    head_weight = np.asarray(head_weight, dtype=np.float32)
    gate = np.asarray(gate, dtype=np.float32).reshape(D)
    soft_gate = np.asarray(soft_gate, dtype=np.float32).reshape(D)

    w2 = (emb_weight.astype(np.float64) @ head_weight.astype(np.float64).T)
    w2 = np.ascontiguousarray(w2.astype(np.float32))
    hw_t = head_weight.T  # [D, V]
    hw_t_sb = np.ascontiguousarray(
        hw_t.reshape(NK, P, V).transpose(1, 0, 2).reshape(P, NK * V))
    gate_rep = np.ascontiguousarray(np.broadcast_to(gate[None, :] * 0.5, (P, D)))
    sgate_cols = np.ascontiguousarray(soft_gate.reshape(NK, P).T) * 0.5  # [P,NK], tanh(z/2) form
    iota64 = np.ascontiguousarray(
        np.broadcast_to(np.arange(P, dtype=np.float32)[None, :], (P, P)))
    import ml_dtypes
    ones = np.ones((P, 1), dtype=ml_dtypes.bfloat16)
    w_emb_b = emb_weight.astype(ml_dtypes.bfloat16)
    w_emb_bl = (emb_weight - w_emb_b.astype(np.float32)).astype(ml_dtypes.bfloat16)
    ident = np.eye(P, dtype=np.float32)

    in_maps = []
    for c in range(N_CORES):
        xs = x[c * TOK:(c + 1) * TOK]
        idxs = idx_c[c * TOK:(c + 1) * TOK]
        idxf2d = np.ascontiguousarray(idxs.reshape(NT, P).T)  # [P, NT]
        in_maps.append({
            "x": xs,
            "idxf": idxf2d,
            "w_emb": emb_weight,
            "hw_t": hw_t_sb,
            "w2": w2,
            "gate_rep": gate_rep,
            "sgate_cols": sgate_cols,
            "iota64": iota64,
            "ones": ones,
            "w_emb_b": w_emb_b,
            "w_emb_bl": w_emb_bl,
            "ident": ident,
        })
    return in_maps


def _split_multiwait(nc, max_waits=1):
    """Walrus codegen for TRN2 encodes at most one sync-wait per compute
    instruction; Tile's sem assignment attaches several. Hoist the extra
    waits onto same-engine ENGINE_NOPs inserted just before the offender
    (per-engine program order makes this equivalent)."""
    for f in nc.m.functions:
        blks = list(f.blocks)
        for blk in blks:
            snapshot = list(blk.instructions)
            rebuilt = []
            made = []
            for ins in snapshot:
                si = ins.sync_info
                if si is not None and si.on_wait and len(si.on_wait) > max_waits:
                    waits = list(si.on_wait)
                    extra, keep = waits[:-max_waits], waits[-max_waits:]
                    for w in extra:
                        nop = nc.engines[ins.engine].nop(nofuse=True).ins
                        nop.sync_info = mybir.SyncInfo(on_wait=[w], on_update=[])
                        made.append(nop.name)
                        rebuilt.append(nop)
                    ins.sync_info = mybir.SyncInfo(
                        on_wait=list(keep), on_update=list(si.on_update or []))
                rebuilt.append(ins)
            if made:
                # engine_nop() appended the new nops to some current bb tail;
                # strip those stray copies from every block, then install.
                made_set = set(made)
                for b2 in blks:
                    cur = list(b2.instructions)
                    keep_list = [i for i in cur if not (i.name in made_set and b2 is not blk)]
                    if b2 is blk:
                        keep_list = rebuilt
                    b2.instructions[:] = keep_list


TRACE = False           # set True (e.g. from test.py) to collect an NTFF profile
LAST_RESULT = {}        # exec_time_ns etc. from the most recent run


def kernel(x, idx, emb_weight, head_weight, gate, soft_gate):
    idx = np.asarray(idx)
    in_maps = _prep_inputs(x, idx, emb_weight, head_weight, gate, soft_gate)
    if "nc" not in _CACHE:
        nc = build_kernel()
        _split_multiwait(nc)
        _CACHE["nc"] = nc
    nc = _CACHE["nc"]
    res = run_bass_kernel_spmd(nc, in_maps, list(range(N_CORES)), trace=TRACE)
    LAST_RESULT["exec_time_ns"] = res.exec_time_ns
    LAST_RESULT["res"] = res
    xs_out = []
    idxs_out = []
    for c in range(N_CORES):
        out = res.results[c]
        xs_out.append(np.asarray(out["x_out"]).reshape(TOK, D))
        idxs_out.append(np.asarray(out["idx_out"]).T.reshape(TOK))
    x_full = np.concatenate(xs_out, axis=0).reshape(B, T, D)
    idx_full = np.concatenate(idxs_out, axis=0).reshape(B, T).astype(idx.dtype)
    return x_full, idx_full


# revision 24
# speedup vs baseline: 1.1513x; 1.1513x over previous
"""Trainium2 Bass kernel for the vq_codebook block (nn_Block_89824946029265).

Computation (per token t, D=1024, V=64):
  g_t  = mean_d sigmoid(x * gate)
  x1   = (1-g) x + g emb_weight[idx]
  logits = x1 @ head_weight.T            # [*, 64]
  soft_prob = softmax(logits)
  x2   = (1-sg) x1 + sg (soft_prob @ emb_weight),  sg = mean_d sigmoid(x1*soft_gate)
  new_idx = argmax(logits)

Reformulation used on device (W2 := emb_weight @ head_weight.T, host f64):
  xs      = (1-g) x
  logits  = xs @ head_weight.T + g*W2[idx]            (exact f32 path -> argmax safe)
  x1^T    = xs^T + (g*one_hot)^T.T-matmul(emb_weight) (PSUM, f32r, feeds sigma2 only)
  sg      = mean sigma(softgate ⊙ x1^T)               (ACT on transposed blocks)
  x2      = (1-sg) xs + [(1-sg)*g*one_hot + sg*soft_prob] @ emb_weight   (f32r)

Sharding: pure data parallel, batch dim B=8 -> one batch row (4096 tokens)
per NeuronCore.
"""

import numpy as np

import concourse.bass as bass
import concourse.mybir as mybir
from concourse import tile
from concourse.bass_utils import run_bass_kernel_spmd
from concourse.tile_rust import add_dep_helper


def _dep(later, earlier):
    """Order `later` after `earlier` (PSUM bank first-writer ordering)."""
    add_dep_helper(later.ins, earlier.ins, reason="psum bank start ordering")

F32 = mybir.dt.float32
F32R = mybir.dt.float32r
I32 = mybir.dt.int32
BF16 = mybir.dt.bfloat16
AX = mybir.AxisListType
OP = mybir.AluOpType
AF = mybir.ActivationFunctionType

N_CORES = 8
B, T, D, V = 8, 4096, 1024, 64
TOK = B * T // N_CORES          # 4096 tokens per core
P = 128                         # partitions / tokens per tile
NT = TOK // P                   # 32 tiles per core
NK = D // P                     # 8 D-slices
NPAIR = NT // 2                 # 16 pairs

_CACHE = {}


def r(ap):
    """View an fp32 AP as float32r (fast PE path, fp22 truncation)."""
    return ap.bitcast(F32R)


def build_kernel():
    nc = bass.Bass()

    # ---- per-core I/O ----
    x_in = nc.declare_dram_parameter("x", [TOK, D], F32, isOutput=False)
    idxf_in = nc.declare_dram_parameter("idxf", [P, NT], F32, isOutput=False)
    wemb_in = nc.declare_dram_parameter("w_emb", [V, D], F32, isOutput=False)
    hwt_in = nc.declare_dram_parameter("hw_t", [P, NK * V], F32, isOutput=False)
    w2_in = nc.declare_dram_parameter("w2", [V, V], F32, isOutput=False)
    gate_in = nc.declare_dram_parameter("gate_rep", [P, D], F32, isOutput=False)
    sgc_in = nc.declare_dram_parameter("sgate_cols", [P, NK], F32, isOutput=False)
    iota_in = nc.declare_dram_parameter("iota64", [P, P], F32, isOutput=False)
    ones_in = nc.declare_dram_parameter("ones", [P, 1], BF16, isOutput=False)
    wembb_in = nc.declare_dram_parameter("w_emb_b", [V, D], BF16, isOutput=False)
    wembbl_in = nc.declare_dram_parameter("w_emb_bl", [V, D], BF16, isOutput=False)
    ident_in = nc.declare_dram_parameter("ident", [P, P], F32, isOutput=False)

    x_out = nc.declare_dram_parameter("x_out", [TOK, D], F32, isOutput=True)
    idx_out = nc.declare_dram_parameter("idx_out", [P, NT], I32, isOutput=True)

    with tile.TileContext(nc) as tc:
        with (
            tc.tile_pool(name="const", bufs=1) as cpool,
            tc.tile_pool(name="io", bufs=4) as iopool,
            tc.tile_pool(name="work", bufs=4) as wpool,
            tc.tile_pool(name="pairw", bufs=4) as ppool,
            tc.tile_pool(name="small", bufs=6) as spool,
            tc.tile_pool(name="ps_big", bufs=2, space="PSUM") as ps_big,
            tc.tile_pool(name="ps_lg", bufs=3, space="PSUM") as ps_lg,
            tc.tile_pool(name="ps_comb", bufs=1, space="PSUM") as ps_comb,
        ):
            # ---- preload constants ----
            w_emb = cpool.tile([V, D], F32)
            wembB = cpool.tile([V, D], BF16)
            wembBlo = cpool.tile([V, D], BF16)
            hw_t = cpool.tile([P, NK * V], F32)
            w2 = cpool.tile([V, V], F32)
            gate_rep = cpool.tile([P, D], F32)
            sgate_cols = cpool.tile([P, NK], F32)
            iota128 = cpool.tile([P, P], F32)
            ones = cpool.tile([P, 1], BF16)
            ident = cpool.tile([P, P], F32)
            idxf = cpool.tile([P, NT], F32)
            idx_sb = cpool.tile([P, NT], I32)

            nc.sync.dma_start(w_emb[:], wemb_in[:])
            nc.sync.dma_start(wembB[:], wembb_in[:])
            nc.sync.dma_start(wembBlo[:], wembbl_in[:])
            nc.sync.dma_start(hw_t[:], hwt_in[:])
            nc.sync.dma_start(w2[:], w2_in[:])
            nc.sync.dma_start(gate_rep[:], gate_in[:])
            nc.sync.dma_start(sgate_cols[:], sgc_in[:])
            nc.sync.dma_start(iota128[:], iota_in[:])
            nc.sync.dma_start(ones[:], ones_in[:])
            nc.sync.dma_start(ident[:], ident_in[:])
            nc.sync.dma_start(idxf[:], idxf_in[:])

            for i in range(NT):
                s0 = i * P
                # ---------- per-tile PSUM ----------
                x1t = ps_big.tile([P, D], F32)        # xs^T then x1^T (2 banks)
                # lgbank regions: logits 0:64 | ohgT 64:192 | sgrow(p0) 192:320
                #                 | pcombT 320:448 | sgcol 448
                lgbank = ps_lg.tile([P, 512], F32)
                cmb_t = ps_comb.tile([P, 512], F32)   # comb (1 bank, late phase)

                # ---------- token-space gate 1 ----------
                x_t = iopool.tile([P, D], F32, tag="x")
                nc.sync.dma_start(x_t[:], x_in[s0:s0 + P, :])

                m1 = wpool.tile([P, D], F32, tag="m1")
                nc.gpsimd.tensor_tensor(m1[:], x_t[:], gate_rep[:], OP.mult)

                s1 = wpool.tile([P, D], BF16, tag="s1")
                gsum = spool.tile([P, 1], F32, tag="gsum")
                nc.scalar.activation(s1[:], m1[:], AF.Tanh, accum_out=gsum[:])

                gcol = spool.tile([P, 1], F32, tag="gcol")
                g1m = spool.tile([P, 1], F32, tag="g1m")
                nc.gpsimd.tensor_scalar(gcol[:], gsum[:], 0.5 / D, 0.5, OP.mult, OP.add)
                nc.gpsimd.tensor_scalar(g1m[:], gsum[:], -0.5 / D, 0.5, OP.mult, OP.add)

                # xs = (1-g) * x  (GPSIMD single-input ~line rate)
                xs = wpool.tile([P, D], F32, tag="xs")
                nc.gpsimd.tensor_scalar(xs[:], x_t[:], g1m[:], None, OP.mult)

                # g-scaled one-hot, padded to 128 cols (64: zeros)
                ohg = spool.tile([P, P], F32, tag="ohg")
                nc.gpsimd.tensor_scalar(
                    ohg[:], iota128[:], idxf[:, i:i + 1], gcol[:],
                    OP.is_equal, OP.mult)
                lg_first = nc.tensor.matmul(
                    lgbank[:, 64:64 + P], ohg[:], ident[:],
                    is_transpose=True, start=True, stop=False,
                    skip_group_check=True)
                ohgT = spool.tile([V, P], F32, tag="ohgT")
                nc.vector.tensor_copy(ohgT[:], lgbank[0:V, 64:64 + P])
                ohgTb = spool.tile([V, P], BF16, tag="ohgTb")
                nc.vector.tensor_copy(ohgTb[:], lgbank[0:V, 64:64 + P])

                # ---------- xs^T transposes + copy + emb accumulation ----------
                bank_first = [None, None]
                for k in range(NK):
                    b = k // 4
                    t = nc.tensor.matmul(
                        x1t[:, k * P:(k + 1) * P],
                        xs[:, k * P:(k + 1) * P], ident[:],
                        is_transpose=True, start=(k % 4 == 0), stop=False,
                        skip_group_check=True)
                    if k % 4 == 0:
                        bank_first[b] = t
                    else:
                        _dep(t, bank_first[b])

                xsT = ppool.tile([P, D], F32, tag="xsT")
                nc.scalar.copy(xsT[:, 0:384], x1t[:, 0:384])
                nc.vector.tensor_copy(xsT[:, 384:D], x1t[:, 384:D])

                # x1^T = xs^T + g*emb^T   (bf16 matmuls, feeds sigma2 only)
                for k in range(NK):
                    nc.tensor.matmul(
                        x1t[:, k * P:(k + 1) * P],
                        wembB[:, k * P:(k + 1) * P], ohgTb[:],
                        start=False, stop=(k % 4 == 3), skip_group_check=True)

                # ---------- sigma2 (tanh form) ----------
                s2t = ppool.tile([P, D], BF16, tag="s2t")
                for k in range(NK):
                    nc.scalar.activation(
                        s2t[:, k * P:(k + 1) * P],
                        x1t[:, k * P:(k + 1) * P],
                        AF.Tanh, scale=sgate_cols[:, k:k + 1])

                # ---------- logits ----------
                lg = lgbank[:, 0:V]
                for k in range(NK):
                    mm = nc.tensor.matmul(
                        lg, xsT[:, k * P:(k + 1) * P],
                        hw_t[:, k * V:(k + 1) * V],
                        start=False, stop=False, skip_group_check=True)
                    if k == 0:
                        _dep(mm, lg_first)
                nc.tensor.matmul(lg, ohgT[:], w2[:],
                                 start=False, stop=True, skip_group_check=True)

                # sg sums (partition 0, cols 192:320)
                for k in range(NK):
                    mm = nc.tensor.matmul(
                        lgbank[0:1, 192:192 + P],
                        ones[:], s2t[:, k * P:(k + 1) * P],
                        start=False, stop=(k == NK - 1),
                        skip_group_check=True)
                    if k == 0:
                        _dep(mm, lg_first)
                sgrow = spool.tile([1, P], F32, tag="sgrow")
                nc.vector.tensor_copy(sgrow[:], lgbank[0:1, 192:192 + P])
                t_sg = nc.tensor.matmul(lgbank[:, 448:449], sgrow[0:1, :],
                                        ident[0:1, 0:1],
                                        is_transpose=True, start=False, stop=True,
                                        skip_group_check=True)
                _dep(t_sg, lg_first)
                sgsum = spool.tile([P, 1], F32, tag="sgsum")
                nc.vector.tensor_copy(sgsum[:], lgbank[:, 448:449])

                # ---------- softmax + argmax ----------
                lgs = spool.tile([P, V], F32, tag="lgs")
                nc.vector.tensor_copy(lgs[:], lg)
                mx8 = spool.tile([P, 8], F32, tag="mx8")
                nc.vector.max(mx8[:], lgs[:])
                ix8 = spool.tile([P, 8], mybir.dt.uint32, tag="ix8")
                nc.vector.max_index(ix8[:], mx8[:], lgs[:])
                nc.vector.tensor_copy(idx_sb[:, i:i + 1], ix8[:, 0:1].bitcast(I32))

                nmx = spool.tile([P, 1], F32, tag="nmx")
                nc.gpsimd.tensor_scalar(nmx[:], mx8[:, 0:1], -1.0, None, OP.mult)
                exps = spool.tile([P, V], F32, tag="exps")
                sume = spool.tile([P, 1], F32, tag="sume")
                nc.scalar.activation(exps[:], lg, AF.Exp, bias=nmx[:],
                                     accum_out=sume[:])
                rcp = spool.tile([P, 1], F32, tag="rcp")
                nc.vector.reciprocal(rcp[:], sume[:])

                sgv = spool.tile([P, 1], F32, tag="sgv")
                nc.gpsimd.tensor_scalar(sgv[:], sgsum[:], 0.5 / D, 0.5,
                                        OP.mult, OP.add)
                alpha = spool.tile([P, 1], F32, tag="alpha")
                nc.gpsimd.tensor_tensor(alpha[:], sgv[:], rcp[:], OP.mult)
                beta = spool.tile([P, 1], F32, tag="beta")
                nc.gpsimd.tensor_scalar(beta[:], sgsum[:], -0.5 / D, 0.5,
                                        OP.mult, OP.add)

                spr = spool.tile([P, V], F32, tag="spr")
                nc.gpsimd.tensor_scalar(spr[:], exps[:], alpha[:], None, OP.mult)
                pcomb = spool.tile([P, V], F32, tag="pcomb")
                nc.vector.scalar_tensor_tensor(pcomb[:], ohg[:, 0:V], beta[:], spr[:],
                                               OP.mult, OP.add)
                t_pc = nc.tensor.matmul(
                    lgbank[0:V, 320:320 + P], pcomb[:], ident[:],
                    is_transpose=True, start=False, stop=True,
                    skip_group_check=True)
                _dep(t_pc, lg_first)
                pcombT = spool.tile([V, P], F32, tag="pcombT")
                nc.vector.tensor_copy(pcombT[:], lgbank[0:V, 320:320 + P])
                pcT_hi = spool.tile([V, P], BF16, tag="pcT_hi")
                nc.vector.tensor_copy(pcT_hi[:], pcombT[:])
                pcT_lo = spool.tile([V, P], BF16, tag="pcT_lo")
                nc.vector.tensor_tensor(pcT_lo[:], pcombT[:], pcT_hi[:],
                                        OP.subtract)

                # ---------- comb + x2 ----------
                x2 = iopool.tile([P, D], F32, tag="x2")
                for h in range(2):
                    if h:
                        cmb = ps_comb.tile([P, 512], F32, tag="cmb_t")
                    else:
                        cmb = cmb_t
                    wh = slice(h * 512, (h + 1) * 512)
                    nc.tensor.matmul(cmb[:], pcT_hi[:], wembB[:, wh],
                                     start=True, stop=False,
                                     skip_group_check=True)
                    nc.tensor.matmul(cmb[:], pcT_lo[:], wembB[:, wh],
                                     start=False, stop=False,
                                     skip_group_check=True)
                    nc.tensor.matmul(cmb[:], pcT_hi[:], wembBlo[:, wh],
                                     start=False, stop=True,
                                     skip_group_check=True)
                    nc.vector.scalar_tensor_tensor(
                        x2[:, wh], xs[:, wh], beta[:], cmb[:], OP.mult, OP.add)
                nc.sync.dma_start(x_out[s0:s0 + P, :], x2[:])

            nc.sync.dma_start(idx_out[:], idx_sb[:])

    return nc


def _prep_inputs(x, idx, emb_weight, head_weight, gate, soft_gate):
    """Host-side prep: shard + derived tensors. Returns per-core input maps."""
    x = np.ascontiguousarray(np.asarray(x, dtype=np.float32).reshape(B * T, D))
    idx = np.asarray(idx)
    idx_c = np.clip(idx, 0, None).astype(np.float32).reshape(B * T)
    emb_weight = np.asarray(emb_weight, dtype=np.float32)
    head_weight = np.asarray(head_weight, dtype=np.float32)
    gate = np.asarray(gate, dtype=np.float32).reshape(D)
    soft_gate = np.asarray(soft_gate, dtype=np.float32).reshape(D)

    w2 = (emb_weight.astype(np.float64) @ head_weight.astype(np.float64).T)
    w2 = np.ascontiguousarray(w2.astype(np.float32))
    hw_t = head_weight.T  # [D, V]
    hw_t_sb = np.ascontiguousarray(
        hw_t.reshape(NK, P, V).transpose(1, 0, 2).reshape(P, NK * V))
    gate_rep = np.ascontiguousarray(np.broadcast_to(gate[None, :] * 0.5, (P, D)))
    sgate_cols = np.ascontiguousarray(soft_gate.reshape(NK, P).T) * 0.5  # [P,NK], tanh(z/2) form
    iota64 = np.ascontiguousarray(
        np.broadcast_to(np.arange(P, dtype=np.float32)[None, :], (P, P)))
    import ml_dtypes
    ones = np.ones((P, 1), dtype=ml_dtypes.bfloat16)
    w_emb_b = emb_weight.astype(ml_dtypes.bfloat16)
    w_emb_bl = (emb_weight - w_emb_b.astype(np.float32)).astype(ml_dtypes.bfloat16)
    ident = np.eye(P, dtype=np.float32)

    in_maps = []
    for c in range(N_CORES):
        xs = x[c * TOK:(c + 1) * TOK]
        idxs = idx_c[c * TOK:(c + 1) * TOK]
        idxf2d = np.ascontiguousarray(idxs.reshape(NT, P).T)  # [P, NT]
        in_maps.append({
            "x": xs,
            "idxf": idxf2d,
            "w_emb": emb_weight,
            "hw_t": hw_t_sb,
            "w2": w2,
            "gate_rep": gate_rep,
            "sgate_cols": sgate_cols,
            "iota64": iota64,
            "ones": ones,
            "w_emb_b": w_emb_b,
            "w_emb_bl": w_emb_bl,
            "ident": ident,
        })
    return in_maps


def _split_multiwait(nc, max_waits=1):
    """Walrus codegen for TRN2 encodes at most one sync-wait per compute
    instruction; Tile's sem assignment attaches several. Hoist the extra
    waits onto same-engine ENGINE_NOPs inserted just before the offender
    (per-engine program order makes this equivalent)."""
    for f in nc.m.functions:
        blks = list(f.blocks)
        for blk in blks:
            snapshot = list(blk.instructions)
            rebuilt = []
            made = []
            for ins in snapshot:
                si = ins.sync_info
                if si is not None and si.on_wait and len(si.on_wait) > max_waits:
                    waits = list(si.on_wait)
                    extra, keep = waits[:-max_waits], waits[-max_waits:]
                    for w in extra:
                        nop = nc.engines[ins.engine].nop(nofuse=True).ins
                        nop.sync_info = mybir.SyncInfo(on_wait=[w], on_update=[])
                        made.append(nop.name)
                        rebuilt.append(nop)
                    ins.sync_info = mybir.SyncInfo(
                        on_wait=list(keep), on_update=list(si.on_update or []))
                rebuilt.append(ins)
            if made:
                # engine_nop() appended the new nops to some current bb tail;
                # strip those stray copies from every block, then install.
                made_set = set(made)
                for b2 in blks:
                    cur = list(b2.instructions)
                    keep_list = [i for i in cur if not (i.name in made_set and b2 is not blk)]
                    if b2 is blk:
                        keep_list = rebuilt
                    b2.instructions[:] = keep_list


TRACE = False           # set True (e.g. from test.py) to collect an NTFF profile
LAST_RESULT = {}        # exec_time_ns etc. from the most recent run


def kernel(x, idx, emb_weight, head_weight, gate, soft_gate):
    idx = np.asarray(idx)
    in_maps = _prep_inputs(x, idx, emb_weight, head_weight, gate, soft_gate)
    if "nc" not in _CACHE:
        nc = build_kernel()
        _split_multiwait(nc)
        _CACHE["nc"] = nc
    nc = _CACHE["nc"]
    res = run_bass_kernel_spmd(nc, in_maps, list(range(N_CORES)), trace=TRACE)
    LAST_RESULT["exec_time_ns"] = res.exec_time_ns
    LAST_RESULT["res"] = res
    xs_out = []
    idxs_out = []
    for c in range(N_CORES):
        out = res.results[c]
        xs_out.append(np.asarray(out["x_out"]).reshape(TOK, D))
        idxs_out.append(np.asarray(out["idx_out"]).T.reshape(TOK))
    x_full = np.concatenate(xs_out, axis=0).reshape(B, T, D)
    idx_full = np.concatenate(idxs_out, axis=0).reshape(B, T).astype(idx.dtype)
    return x_full, idx_full


# revision 25
# speedup vs baseline: 1.1894x; 1.0332x over previous
"""Trainium2 Bass kernel for the vq_codebook block (nn_Block_89824946029265).

Computation (per token t, D=1024, V=64):
  g_t  = mean_d sigmoid(x * gate)
  x1   = (1-g) x + g emb_weight[idx]
  logits = x1 @ head_weight.T            # [*, 64]
  soft_prob = softmax(logits)
  x2   = (1-sg) x1 + sg (soft_prob @ emb_weight),  sg = mean_d sigmoid(x1*soft_gate)
  new_idx = argmax(logits)

Reformulation used on device (W2 := emb_weight @ head_weight.T, host f64):
  xs      = (1-g) x
  logits  = xs @ head_weight.T + g*W2[idx]            (exact f32 path -> argmax safe)
  x1^T    = xs^T + (g*one_hot)^T.T-matmul(emb_weight) (PSUM, f32r, feeds sigma2 only)
  sg      = mean sigma(softgate ⊙ x1^T)               (ACT on transposed blocks)
  x2      = (1-sg) xs + [(1-sg)*g*one_hot + sg*soft_prob] @ emb_weight   (f32r)

Sharding: pure data parallel, batch dim B=8 -> one batch row (4096 tokens)
per NeuronCore.
"""

import numpy as np

import concourse.bass as bass
import concourse.mybir as mybir
from concourse import tile
from concourse.bass_utils import run_bass_kernel_spmd
from concourse.tile_rust import add_dep_helper


def _dep(later, earlier):
    """Order `later` after `earlier` (PSUM bank first-writer ordering)."""
    add_dep_helper(later.ins, earlier.ins, reason="psum bank start ordering")

F32 = mybir.dt.float32
F32R = mybir.dt.float32r
I32 = mybir.dt.int32
BF16 = mybir.dt.bfloat16
AX = mybir.AxisListType
OP = mybir.AluOpType
AF = mybir.ActivationFunctionType

N_CORES = 8
B, T, D, V = 8, 4096, 1024, 64
TOK = B * T // N_CORES          # 4096 tokens per core
P = 128                         # partitions / tokens per tile
NT = TOK // P                   # 32 tiles per core
NK = D // P                     # 8 D-slices
NPAIR = NT // 2                 # 16 pairs

_CACHE = {}


def r(ap):
    """View an fp32 AP as float32r (fast PE path, fp22 truncation)."""
    return ap.bitcast(F32R)


def build_kernel():
    nc = bass.Bass()

    # ---- per-core I/O ----
    x_in = nc.declare_dram_parameter("x", [TOK, D], F32, isOutput=False)
    idxf_in = nc.declare_dram_parameter("idxf", [P, NT], F32, isOutput=False)
    wemb_in = nc.declare_dram_parameter("w_emb", [V, D], F32, isOutput=False)
    hwt_in = nc.declare_dram_parameter("hw_t", [P, NK * V], F32, isOutput=False)
    w2_in = nc.declare_dram_parameter("w2", [V, V], F32, isOutput=False)
    gate_in = nc.declare_dram_parameter("gate_rep", [P, D], F32, isOutput=False)
    sgr_in = nc.declare_dram_parameter("sgate_rep", [P, D], F32, isOutput=False)
    wembsg_in = nc.declare_dram_parameter("w_emb_sg", [V, D], BF16, isOutput=False)
    iota_in = nc.declare_dram_parameter("iota64", [P, P], F32, isOutput=False)
    ones_in = nc.declare_dram_parameter("ones", [P, 1], BF16, isOutput=False)
    wembb_in = nc.declare_dram_parameter("w_emb_b", [V, D], BF16, isOutput=False)
    wembbl_in = nc.declare_dram_parameter("w_emb_bl", [V, D], BF16, isOutput=False)
    ident_in = nc.declare_dram_parameter("ident", [P, P], F32, isOutput=False)

    x_out = nc.declare_dram_parameter("x_out", [TOK, D], F32, isOutput=True)
    idx_out = nc.declare_dram_parameter("idx_out", [P, NT], I32, isOutput=True)

    with tile.TileContext(nc) as tc:
        with (
            tc.tile_pool(name="const", bufs=1) as cpool,
            tc.tile_pool(name="io", bufs=4) as iopool,
            tc.tile_pool(name="work", bufs=4) as wpool,
            tc.tile_pool(name="pairw", bufs=4) as ppool,
            tc.tile_pool(name="small", bufs=6) as spool,
            tc.tile_pool(name="ps_big", bufs=2, space="PSUM") as ps_big,
            tc.tile_pool(name="ps_lg", bufs=3, space="PSUM") as ps_lg,
            tc.tile_pool(name="ps_comb", bufs=1, space="PSUM") as ps_comb,
        ):
            # ---- preload constants ----
            w_emb = cpool.tile([V, D], F32)
            wembB = cpool.tile([V, D], BF16)
            wembBlo = cpool.tile([V, D], BF16)
            hw_t = cpool.tile([P, NK * V], F32)
            w2 = cpool.tile([V, V], F32)
            gate_rep = cpool.tile([P, D], F32)
            sgate_rep = cpool.tile([P, D], F32)
            wembSg = cpool.tile([V, D], BF16)
            iota128 = cpool.tile([P, P], F32)
            ones = cpool.tile([P, 1], BF16)
            ident = cpool.tile([P, P], F32)
            idxf = cpool.tile([P, NT], F32)
            idx_sb = cpool.tile([P, NT], I32)

            nc.sync.dma_start(w_emb[:], wemb_in[:])
            nc.sync.dma_start(wembB[:], wembb_in[:])
            nc.sync.dma_start(wembBlo[:], wembbl_in[:])
            nc.sync.dma_start(hw_t[:], hwt_in[:])
            nc.sync.dma_start(w2[:], w2_in[:])
            nc.sync.dma_start(gate_rep[:], gate_in[:])
            nc.sync.dma_start(sgate_rep[:], sgr_in[:])
            nc.sync.dma_start(wembSg[:], wembsg_in[:])
            nc.sync.dma_start(iota128[:], iota_in[:])
            nc.sync.dma_start(ones[:], ones_in[:])
            nc.sync.dma_start(ident[:], ident_in[:])
            nc.sync.dma_start(idxf[:], idxf_in[:])

            for i in range(NT):
                s0 = i * P
                # ---------- per-tile PSUM ----------
                x1t = ps_big.tile([P, D], F32)        # xs^T then x1^T (2 banks)
                # lgbank regions: logits 0:64 | ohgT 64:192 | sgrow(p0) 192:320
                #                 | pcombT 320:448 | sgcol 448
                lgbank = ps_lg.tile([P, 512], F32)
                cmb_t = ps_comb.tile([P, 512], F32)   # comb (1 bank, late phase)

                # ---------- token-space gate 1 ----------
                x_t = iopool.tile([P, D], F32, tag="x")
                nc.sync.dma_start(x_t[:], x_in[s0:s0 + P, :])

                m1 = wpool.tile([P, D], F32, tag="m1")
                nc.gpsimd.tensor_tensor(m1[:], x_t[:], gate_rep[:], OP.mult)

                s1 = wpool.tile([P, D], BF16, tag="s1")
                gsum = spool.tile([P, 1], F32, tag="gsum")
                nc.scalar.activation(s1[:], m1[:], AF.Tanh, accum_out=gsum[:])

                gcol = spool.tile([P, 1], F32, tag="gcol")
                g1m = spool.tile([P, 1], F32, tag="g1m")
                nc.gpsimd.tensor_scalar(gcol[:], gsum[:], 0.5 / D, 0.5, OP.mult, OP.add)
                nc.gpsimd.tensor_scalar(g1m[:], gsum[:], -0.5 / D, 0.5, OP.mult, OP.add)

                # xs = (1-g) * x  (GPSIMD single-input ~line rate)
                xs = wpool.tile([P, D], F32, tag="xs")
                nc.gpsimd.tensor_scalar(xs[:], x_t[:], g1m[:], None, OP.mult)

                # xsg = xs * softgate/2  (pre-scaled sigma2/logits operand)
                xsg = wpool.tile([P, D], F32, tag="xsg")
                nc.gpsimd.tensor_tensor(xsg[:], xs[:], sgate_rep[:], OP.mult)

                # g-scaled one-hot, padded to 128 cols (64: zeros)
                ohg = spool.tile([P, P], F32, tag="ohg")
                nc.gpsimd.tensor_scalar(
                    ohg[:], iota128[:], idxf[:, i:i + 1], gcol[:],
                    OP.is_equal, OP.mult)
                lg_first = nc.tensor.matmul(
                    lgbank[:, 64:64 + P], ohg[:], ident[:],
                    is_transpose=True, start=True, stop=False,
                    skip_group_check=True)
                ohgT = spool.tile([V, P], F32, tag="ohgT")
                nc.vector.tensor_copy(ohgT[:], lgbank[0:V, 64:64 + P])
                ohgTb = spool.tile([V, P], BF16, tag="ohgTb")
                nc.vector.tensor_copy(ohgTb[:], lgbank[0:V, 64:64 + P])

                # ---------- xs^T transposes + copy + emb accumulation ----------
                bank_first = [None, None]
                for k in range(NK):
                    b = k // 4
                    t = nc.tensor.matmul(
                        x1t[:, k * P:(k + 1) * P],
                        xsg[:, k * P:(k + 1) * P], ident[:],
                        is_transpose=True, start=(k % 4 == 0), stop=False,
                        skip_group_check=True)
                    if k % 4 == 0:
                        bank_first[b] = t
                    else:
                        _dep(t, bank_first[b])

                xsT = ppool.tile([P, D], F32, tag="xsT")
                nc.scalar.copy(xsT[:, 0:384], x1t[:, 0:384])
                nc.vector.tensor_copy(xsT[:, 384:D], x1t[:, 384:D])

                # x1^T = xs^T + g*emb^T   (bf16 matmuls, feeds sigma2 only)
                for k in range(NK):
                    nc.tensor.matmul(
                        x1t[:, k * P:(k + 1) * P],
                        wembSg[:, k * P:(k + 1) * P], ohgTb[:],
                        start=False, stop=(k % 4 == 3), skip_group_check=True)

                # ---------- sigma2 (tanh form) ----------
                s2t = ppool.tile([P, D], BF16, tag="s2t")
                nc.scalar.activation(s2t[:], x1t[:], AF.Tanh)

                # ---------- logits ----------
                lg = lgbank[:, 0:V]
                for k in range(NK):
                    mm = nc.tensor.matmul(
                        lg, xsT[:, k * P:(k + 1) * P],
                        hw_t[:, k * V:(k + 1) * V],
                        start=False, stop=False, skip_group_check=True)
                    if k == 0:
                        _dep(mm, lg_first)
                nc.tensor.matmul(lg, ohgT[:], w2[:],
                                 start=False, stop=True, skip_group_check=True)

                # sg sums (partition 0, cols 192:320)
                for k in range(NK):
                    mm = nc.tensor.matmul(
                        lgbank[0:1, 192:192 + P],
                        ones[:], s2t[:, k * P:(k + 1) * P],
                        start=False, stop=(k == NK - 1),
                        skip_group_check=True)
                    if k == 0:
                        _dep(mm, lg_first)
                sgrow = spool.tile([1, P], F32, tag="sgrow")
                nc.vector.tensor_copy(sgrow[:], lgbank[0:1, 192:192 + P])
                t_sg = nc.tensor.matmul(lgbank[:, 448:449], sgrow[0:1, :],
                                        ident[0:1, 0:1],
                                        is_transpose=True, start=False, stop=True,
                                        skip_group_check=True)
                _dep(t_sg, lg_first)
                sgsum = spool.tile([P, 1], F32, tag="sgsum")
                nc.vector.tensor_copy(sgsum[:], lgbank[:, 448:449])

                # ---------- softmax + argmax ----------
                lgs = spool.tile([P, V], F32, tag="lgs")
                nc.vector.tensor_copy(lgs[:], lg)
                mx8 = spool.tile([P, 8], F32, tag="mx8")
                nc.vector.max(mx8[:], lgs[:])
                ix8 = spool.tile([P, 8], mybir.dt.uint32, tag="ix8")
                nc.vector.max_index(ix8[:], mx8[:], lgs[:])
                nc.vector.tensor_copy(idx_sb[:, i:i + 1], ix8[:, 0:1].bitcast(I32))

                nmx = spool.tile([P, 1], F32, tag="nmx")
                nc.gpsimd.tensor_scalar(nmx[:], mx8[:, 0:1], -1.0, None, OP.mult)
                exps = spool.tile([P, V], F32, tag="exps")
                sume = spool.tile([P, 1], F32, tag="sume")
                nc.scalar.activation(exps[:], lg, AF.Exp, bias=nmx[:],
                                     accum_out=sume[:])
                rcp = spool.tile([P, 1], F32, tag="rcp")
                nc.vector.reciprocal(rcp[:], sume[:])

                sgv = spool.tile([P, 1], F32, tag="sgv")
                nc.gpsimd.tensor_scalar(sgv[:], sgsum[:], 0.5 / D, 0.5,
                                        OP.mult, OP.add)
                alpha = spool.tile([P, 1], F32, tag="alpha")
                nc.gpsimd.tensor_tensor(alpha[:], sgv[:], rcp[:], OP.mult)
                beta = spool.tile([P, 1], F32, tag="beta")
                nc.gpsimd.tensor_scalar(beta[:], sgsum[:], -0.5 / D, 0.5,
                                        OP.mult, OP.add)

                spr = spool.tile([P, V], F32, tag="spr")
                nc.gpsimd.tensor_scalar(spr[:], exps[:], alpha[:], None, OP.mult)
                pcomb = spool.tile([P, V], F32, tag="pcomb")
                nc.vector.scalar_tensor_tensor(pcomb[:], ohg[:, 0:V], beta[:], spr[:],
                                               OP.mult, OP.add)
                t_pc = nc.tensor.matmul(
                    lgbank[0:V, 320:320 + P], pcomb[:], ident[:],
                    is_transpose=True, start=False, stop=True,
                    skip_group_check=True)
                _dep(t_pc, lg_first)
                pcombT = spool.tile([V, P], F32, tag="pcombT")
                nc.vector.tensor_copy(pcombT[:], lgbank[0:V, 320:320 + P])
                pcT_hi = spool.tile([V, P], BF16, tag="pcT_hi")
                nc.vector.tensor_copy(pcT_hi[:], pcombT[:])
                pcT_lo = spool.tile([V, P], BF16, tag="pcT_lo")
                nc.vector.tensor_tensor(pcT_lo[:], pcombT[:], pcT_hi[:],
                                        OP.subtract)

                # ---------- comb + x2 ----------
                x2 = iopool.tile([P, D], F32, tag="x2")
                for h in range(2):
                    if h:
                        cmb = ps_comb.tile([P, 512], F32, tag="cmb_t")
                    else:
                        cmb = cmb_t
                    wh = slice(h * 512, (h + 1) * 512)
                    nc.tensor.matmul(cmb[:], pcT_hi[:], wembB[:, wh],
                                     start=True, stop=False,
                                     skip_group_check=True)
                    nc.tensor.matmul(cmb[:], pcT_lo[:], wembB[:, wh],
                                     start=False, stop=False,
                                     skip_group_check=True)
                    nc.tensor.matmul(cmb[:], pcT_hi[:], wembBlo[:, wh],
                                     start=False, stop=True,
                                     skip_group_check=True)
                    nc.vector.scalar_tensor_tensor(
                        x2[:, wh], xs[:, wh], beta[:], cmb[:], OP.mult, OP.add)
                nc.sync.dma_start(x_out[s0:s0 + P, :], x2[:])

            nc.sync.dma_start(idx_out[:], idx_sb[:])

    return nc


def _prep_inputs(x, idx, emb_weight, head_weight, gate, soft_gate):
    """Host-side prep: shard + derived tensors. Returns per-core input maps."""
    x = np.ascontiguousarray(np.asarray(x, dtype=np.float32).reshape(B * T, D))
    idx = np.asarray(idx)
    idx_c = np.clip(idx, 0, None).astype(np.float32).reshape(B * T)
    emb_weight = np.asarray(emb_weight, dtype=np.float32)
    head_weight = np.asarray(head_weight, dtype=np.float32)
    gate = np.asarray(gate, dtype=np.float32).reshape(D)
    soft_gate = np.asarray(soft_gate, dtype=np.float32).reshape(D)

    w2 = (emb_weight.astype(np.float64) @ head_weight.astype(np.float64).T)
    w2 = np.ascontiguousarray(w2.astype(np.float32))
    sgate2 = soft_gate * 0.5
    hw_t = head_weight.T / sgate2[:, None]  # [D, V], un-scales the xsg operand
    hw_t_sb = np.ascontiguousarray(
        hw_t.reshape(NK, P, V).transpose(1, 0, 2).reshape(P, NK * V)).astype(np.float32)
    gate_rep = np.ascontiguousarray(np.broadcast_to(gate[None, :] * 0.5, (P, D)))
    sgate_rep = np.ascontiguousarray(np.broadcast_to(sgate2[None, :], (P, D)))
    iota64 = np.ascontiguousarray(
        np.broadcast_to(np.arange(P, dtype=np.float32)[None, :], (P, P)))
    import ml_dtypes
    ones = np.ones((P, 1), dtype=ml_dtypes.bfloat16)
    w_emb_b = emb_weight.astype(ml_dtypes.bfloat16)
    w_emb_sg = (emb_weight * sgate2[None, :]).astype(ml_dtypes.bfloat16)
    w_emb_bl = (emb_weight - w_emb_b.astype(np.float32)).astype(ml_dtypes.bfloat16)
    ident = np.eye(P, dtype=np.float32)

    in_maps = []
    for c in range(N_CORES):
        xs = x[c * TOK:(c + 1) * TOK]
        idxs = idx_c[c * TOK:(c + 1) * TOK]
        idxf2d = np.ascontiguousarray(idxs.reshape(NT, P).T)  # [P, NT]
        in_maps.append({
            "x": xs,
            "idxf": idxf2d,
            "w_emb": emb_weight,
            "hw_t": hw_t_sb,
            "w2": w2,
            "gate_rep": gate_rep,
            "sgate_rep": sgate_rep,
            "w_emb_sg": w_emb_sg,
            "iota64": iota64,
            "ones": ones,
            "w_emb_b": w_emb_b,
            "w_emb_bl": w_emb_bl,
            "ident": ident,
        })
    return in_maps


def _split_multiwait(nc, max_waits=1):
    """Walrus codegen for TRN2 encodes at most one sync-wait per compute
    instruction; Tile's sem assignment attaches several. Hoist the extra
    waits onto same-engine ENGINE_NOPs inserted just before the offender
    (per-engine program order makes this equivalent)."""
    for f in nc.m.functions:
        blks = list(f.blocks)
        for blk in blks:
            snapshot = list(blk.instructions)
            rebuilt = []
            made = []
            for ins in snapshot:
                si = ins.sync_info
                if si is not None and si.on_wait and len(si.on_wait) > max_waits:
                    waits = list(si.on_wait)
                    extra, keep = waits[:-max_waits], waits[-max_waits:]
                    for w in extra:
                        nop = nc.engines[ins.engine].nop(nofuse=True).ins
                        nop.sync_info = mybir.SyncInfo(on_wait=[w], on_update=[])
                        made.append(nop.name)
                        rebuilt.append(nop)
                    ins.sync_info = mybir.SyncInfo(
                        on_wait=list(keep), on_update=list(si.on_update or []))
                rebuilt.append(ins)
            if made:
                # engine_nop() appended the new nops to some current bb tail;
                # strip those stray copies from every block, then install.
                made_set = set(made)
                for b2 in blks:
                    cur = list(b2.instructions)
                    keep_list = [i for i in cur if not (i.name in made_set and b2 is not blk)]
                    if b2 is blk:
                        keep_list = rebuilt
                    b2.instructions[:] = keep_list


TRACE = False           # set True (e.g. from test.py) to collect an NTFF profile
LAST_RESULT = {}        # exec_time_ns etc. from the most recent run


def kernel(x, idx, emb_weight, head_weight, gate, soft_gate):
    idx = np.asarray(idx)
    in_maps = _prep_inputs(x, idx, emb_weight, head_weight, gate, soft_gate)
    if "nc" not in _CACHE:
        nc = build_kernel()
        _split_multiwait(nc)
        _CACHE["nc"] = nc
    nc = _CACHE["nc"]
    res = run_bass_kernel_spmd(nc, in_maps, list(range(N_CORES)), trace=TRACE)
    LAST_RESULT["exec_time_ns"] = res.exec_time_ns
    LAST_RESULT["res"] = res
    xs_out = []
    idxs_out = []
    for c in range(N_CORES):
        out = res.results[c]
        xs_out.append(np.asarray(out["x_out"]).reshape(TOK, D))
        idxs_out.append(np.asarray(out["idx_out"]).T.reshape(TOK))
    x_full = np.concatenate(xs_out, axis=0).reshape(B, T, D)
    idx_full = np.concatenate(idxs_out, axis=0).reshape(B, T).astype(idx.dtype)
    return x_full, idx_full
